# revision 22
# baseline (speedup 1.0000x reference)
"""Trainium2 Bass kernel for nn_BHW2AttnRNNDecoder.

Model (per reference.py):
  B=32, SRC=128, T=64, H=512, V=32000
  - embedding gather of targets
  - precomputable: enc_proj = enc @ Wa_e^T (time invariant)
                   GX = targets @ W_ih_x^T (depends only on embeddings)
  - sequential recurrence over T: GRU step + Bahdanau-style attention
  - big head GEMM logits = [h;ctx] @ Wh^T (+bh) batched over all T at the end

Sharding: pure data-parallel over batch across 8 cores (B_local=4/core),
no collectives. Params replicated. Head streams the (host-pre-tiled) Wh^T
from HBM. Host does embedding gather, transposes/layout prep, final
concat + bh add.

Layout convention on-chip: feature-on-partition ("transposed") so that
small-batch elementwise work uses all 128 lanes.

All tensors fp32.
"""

import os
import numpy as np

import concourse.bass as bass
import concourse.bacc as bacc
import concourse.mybir as mybir
from concourse import tile
from concourse.bass_utils import run_bass_kernel_spmd

F32 = mybir.dt.float32
AF = mybir.ActivationFunctionType
ALU = mybir.AluOpType

B, SRC, T, H, V = 32, 128, 64, 512, 32000
NCORES = 8
BL = B // NCORES          # 4 batches per core
HC = H // 128             # 4 h-chunks
GM = (3 * H) // 128       # 12 gate-chunks
KC2H = (2 * H) // 128     # 8 chunks of [h;ctx]
VP = 32256                # padded vocab (63 * 512)
NB = VP // 512            # 63 vocab tiles
NBLK = 4                  # vocab tiles per head block (PSUM: 2 Mt x 4 Nb = 8 banks)
MT = (BL * T) // 128      # 2 row-tiles for the head (256 rows)


def build_nc(t_steps=T, with_head=True, head_bf16=False, rec_bf16=False):
    BF16 = mybir.dt.bfloat16
    HDT = BF16 if head_bf16 else F32
    RDT = BF16 if rec_bf16 else F32
    nc = bacc.Bacc()

    # ---- I/O ----
    d_h0T = nc.declare_dram_parameter("h0T", [H, BL], F32, isOutput=False)
    d_encT = nc.declare_dram_parameter("encT", [H, BL * SRC], RDT, isOutput=False)
    d_encsb = nc.declare_dram_parameter("encsb", [BL, SRC, H], RDT, isOutput=False)
    d_targT = nc.declare_dram_parameter("targT", [H, BL * T], RDT, isOutput=False)
    d_wihxT = nc.declare_dram_parameter("wihxT", [H, 3 * H], RDT, isOutput=False)
    d_wihcT = nc.declare_dram_parameter("wihcT", [H, 3 * H], RDT, isOutput=False)
    d_whhT = nc.declare_dram_parameter("whhT", [H, 3 * H], RDT, isOutput=False)
    d_wahT = nc.declare_dram_parameter("wahT", [H, H], RDT, isOutput=False)
    d_waeT = nc.declare_dram_parameter("waeT", [H, H], RDT, isOutput=False)
    d_vT = nc.declare_dram_parameter("vT", [H, 1], RDT, isOutput=False)
    d_baT = nc.declare_dram_parameter("baT", [128, HC], F32, isOutput=False)
    d_sones = nc.declare_dram_parameter("sones", [BL, BL * SRC], RDT, isOutput=False)
    d_iden = nc.declare_dram_parameter("iden", [128, 128], RDT, isOutput=False)
    d_bfold = nc.declare_dram_parameter("bfold", [128, GM], F32, isOutput=False)
    d_bhhn = nc.declare_dram_parameter("bhhn", [128, HC], F32, isOutput=False)
    if with_head:
        d_wht = nc.declare_dram_parameter(
            "whtile", [NB, KC2H, 128, 512], HDT, isOutput=False
        )
        d_logits = nc.declare_dram_parameter(
            "logitsP", [BL * T, VP], F32, isOutput=True
        )
    d_attw = nc.declare_dram_parameter("attW", [128, T * BL], F32, isOutput=True)
    d_hfin = nc.declare_dram_parameter("hfin", [128, HC * BL], F32, isOutput=True)
    if t_steps < T:
        d_xtdbg = nc.declare_dram_parameter("xtdbg", [128, KC2H * (T + 1) * BL],
                                            F32, isOutput=True)

    with tile.TileContext(nc) as tc:
        with (
            tc.tile_pool(name="const", bufs=1) as cp,
            tc.tile_pool(name="work", bufs=2) as wp,
        ):
            # ---- load resident tensors ----
            wihc = cp.tile([128, HC * 3 * H], RDT)   # (kc, gate-col)
            nc.sync.dma_start(
                wihc[:].rearrange("p (k c) -> p k c", k=HC),
                d_wihcT[:].rearrange("(k p) c -> p k c", p=128),
            )
            whh = cp.tile([128, HC * 3 * H], RDT)
            nc.sync.dma_start(
                whh[:].rearrange("p (k c) -> p k c", k=HC),
                d_whhT[:].rearrange("(k p) c -> p k c", p=128),
            )
            wah = cp.tile([128, HC * H], RDT)
            nc.sync.dma_start(
                wah[:].rearrange("p (k c) -> p k c", k=HC),
                d_wahT[:].rearrange("(k p) c -> p k c", p=128),
            )
            vt = cp.tile([128, HC], RDT)
            nc.sync.dma_start(
                vt[:].rearrange("p (k o) -> p k o", k=HC),
                d_vT[:].rearrange("(k p) o -> p k o", p=128),
            )
            baT = cp.tile([128, HC], F32)
            nc.sync.dma_start(baT[:], d_baT[:])
            sones = cp.tile([BL, BL * SRC], RDT)
            nc.sync.dma_start(sones[:], d_sones[:])
            iden = cp.tile([128, 128], RDT)
            nc.sync.dma_start(iden[:], d_iden[:])
            bfold = cp.tile([128, GM], F32)
            nc.sync.dma_start(bfold[:], d_bfold[:])
            bhhn = cp.tile([128, HC], F32)
            nc.sync.dma_start(bhhn[:], d_bhhn[:])
            encsb = cp.tile([128, BL * H], RDT)      # (b, h), partition = s
            nc.sync.dma_start(
                encsb[:].rearrange("p (b h) -> p b h", b=BL),
                d_encsb[:].rearrange("b s h -> s b h"),
            )
            # big state buffers
            XT = cp.tile([128, KC2H * (T + 1) * BL], F32)  # (kc, t, b), t in 0..T
            XTr = XT[:].rearrange("p (k t b) -> p k t b", k=KC2H, b=BL)
            attT = cp.tile([128, T * BL], F32)             # (t, b), partition = s
            gxb = cp.tile([128, GM * T * BL], F32)         # (mt, (t,b))
            gxbr = gxb[:].rearrange("p (m c) -> p m c", m=GM)
            epjT = cp.tile([128, HC * BL * SRC], RDT)      # (hc, (b,s)), +ba
            if rec_bf16:
                XTB = cp.tile([128, KC2H * (T + 1) * BL], BF16)
                XTBr = XTB[:].rearrange("p (k t b) -> p k t b", k=KC2H, b=BL)
                attB = cp.tile([128, T * BL], BF16)
            else:
                XTB, XTBr, attB = XT, XTr, attT

            if t_steps < T:  # debug builds: zero-fill unwritten state
                nc.vector.memset(XT[:], 0.0)
                nc.vector.memset(attT[:], 0.0)
                if rec_bf16:
                    nc.vector.memset(XTB[:], 0.0)
                    nc.vector.memset(attB[:], 0.0)

            # h0 -> XT[:, 0:HC, 0, :]
            nc.sync.dma_start(
                XTr[:, 0:HC, 0, :],
                d_h0T[:].rearrange("(k p) b -> p k b", p=128),
            )
            if rec_bf16:
                nc.vector.tensor_copy(XTBr[:, 0:HC, 0, :], XTr[:, 0:HC, 0, :])

            # ================= precompute =================
            with (
                tc.tile_pool(name="pre_psum", bufs=2, space="PSUM") as pp,
                tc.tile_pool(name="pre_sbuf", bufs=1) as pre,
            ):
                wae = pre.tile([128, HC * H], RDT)
                nc.sync.dma_start(
                    wae[:].rearrange("p (k c) -> p k c", k=HC),
                    d_waeT[:].rearrange("(k p) c -> p k c", p=128),
                )
                wihx = pre.tile([128, HC * 3 * H], RDT)
                nc.sync.dma_start(
                    wihx[:].rearrange("p (k c) -> p k c", k=HC),
                    d_wihxT[:].rearrange("(k p) c -> p k c", p=128),
                )
                encT = pre.tile([128, HC * BL * SRC], RDT)  # (kc, (b,s))
                nc.sync.dma_start(
                    encT[:].rearrange("p (k c) -> p k c", k=HC),
                    d_encT[:].rearrange("(k p) c -> p k c", p=128),
                )
                targ = pre.tile([128, HC * BL * T], RDT)  # (kc, (t,b))
                nc.sync.dma_start(
                    targ[:].rearrange("p (k c) -> p k c", k=HC),
                    d_targT[:].rearrange("(k p) c -> p k c", p=128),
                )
                # enc_proj^T [hc, (b,s)] = Wa_e @ enc^T
                for mt in range(HC):
                    ep = pp.tile([128, BL * SRC], F32, tag="ep")
                    for kc in range(HC):
                        nc.tensor.matmul(
                            ep[:],
                            wae[:, kc * H + mt * 128:kc * H + (mt + 1) * 128],
                            encT[:, kc * BL * SRC:(kc + 1) * BL * SRC],
                            start=(kc == 0),
                            stop=(kc == HC - 1),
                        )
                    nc.vector.tensor_scalar_add(
                        epjT[:, mt * BL * SRC:(mt + 1) * BL * SRC], ep[:],
                        baT[:, mt:mt + 1],
                    )
                # GXb [mt, (t,b)] = W_ih_x @ targets^T + bias_fold
                for mt in range(GM):
                    gx = pp.tile([128, BL * T], F32, tag="gx")
                    for kc in range(HC):
                        nc.tensor.matmul(
                            gx[:],
                            wihx[:, kc * 3 * H + mt * 128:kc * 3 * H + (mt + 1) * 128],
                            targ[:, kc * BL * T:(kc + 1) * BL * T],
                            start=(kc == 0),
                            stop=(kc == HC - 1),
                        )
                    nc.vector.tensor_scalar(
                        gxb[:, mt * BL * T:(mt + 1) * BL * T],
                        gx[:],
                        bfold[:, mt:mt + 1],
                        None,
                        op0=ALU.add,
                    )

            # ================= recurrence =================
            with tc.tile_pool(name="sp", bufs=1, space="PSUM") as sp:

                def attention(t):
                    """Attention using h at XT[:, :, t, :]; writes ctx into
                    XT[:, HC:, t, :]; writes normalized w into attT col (t-1)
                    (skipped for t==0)."""
                    # q row-major: q[b, k] = h_b . Wa_h[k, :]
                    qrp = sp.tile([BL, H], F32, tag="qrp")
                    for kc in range(HC):
                        nc.tensor.matmul(
                            qrp[:],
                            XTBr[:, kc, t, :],
                            wah[:, kc * H:(kc + 1) * H],
                            start=(kc == 0),
                            stop=(kc == HC - 1),
                        )
                    qrow = wp.tile([BL, H], RDT, tag="qrow")
                    nc.vector.tensor_copy(qrow[:], qrp[:])
                    # E_pre[hc] = encproj+ba (identity mm) + q broadcast over s
                    ET = wp.tile([128, HC * BL * SRC], RDT, tag="ET")
                    scor = sp.tile([1, BL * SRC], F32, tag="scor")
                    for hc in range(HC):
                        eps = sp.tile([128, BL * SRC], F32, tag="eps", bufs=2)
                        nc.tensor.matmul(
                            eps[:],
                            iden[:],
                            epjT[:, hc * BL * SRC:(hc + 1) * BL * SRC],
                            start=True,
                            stop=False,
                        )
                        nc.tensor.matmul(
                            eps[:],
                            qrow[:, hc * 128:(hc + 1) * 128],
                            sones[:],
                            start=False,
                            stop=True,
                        )
                        nc.scalar.activation(
                            ET[:, hc * BL * SRC:(hc + 1) * BL * SRC],
                            eps[:],
                            AF.Tanh,
                        )
                        # scores accumulate per hc
                        nc.tensor.matmul(
                            scor[:],
                            vt[:, hc:hc + 1],
                            ET[:, hc * BL * SRC:(hc + 1) * BL * SRC],
                            start=(hc == 0),
                            stop=(hc == HC - 1),
                        )
                    # softmax (no max-sub; scores are O(1))
                    wexp = wp.tile([1, BL * SRC], F32, tag="wexp")
                    sums = wp.tile([1, BL], F32, tag="sums")
                    for b in range(BL):
                        nc.scalar.activation(
                            wexp[:, b * SRC:(b + 1) * SRC],
                            scor[:, b * SRC:(b + 1) * SRC],
                            AF.Exp,
                            accum_out=sums[:, b:b + 1],
                        )
                    recip = wp.tile([1, BL], F32, tag="recip")
                    nc.vector.reciprocal(recip[:], sums[:])
                    # wT normalized via K=1 matmul transpose trick
                    wtp = sp.tile([128, BL], F32, tag="wtp")
                    for b in range(BL):
                        nc.tensor.matmul(
                            wtp[:, b:b + 1],
                            wexp[:, b * SRC:(b + 1) * SRC],
                            recip[:, b:b + 1],
                            start=True,
                            stop=True,
                        )
                    if t == 0:
                        wsb = wp.tile([128, BL], F32, tag="wsb")
                        wloc = wsb[:]
                        if rec_bf16:
                            wsbB = wp.tile([128, BL], RDT, tag="wsbB")
                            wlocB = wsbB[:]
                        else:
                            wlocB = wloc
                    else:
                        wloc = attT[:, (t - 1) * BL:t * BL]
                        wlocB = attB[:, (t - 1) * BL:t * BL]
                    nc.vector.tensor_copy(wloc, wtp[:])
                    if rec_bf16:
                        nc.vector.tensor_copy(wlocB, wtp[:])
                    # ctx^T [hc, b] = enc_b^T @ w_b
                    ctxp = sp.tile([128, HC * BL], F32, tag="ctxp")
                    for hc in range(HC):
                        for b in range(BL):
                            nc.tensor.matmul(
                                ctxp[:, hc * BL + b:hc * BL + b + 1],
                                encsb[:, b * H + hc * 128:b * H + (hc + 1) * 128],
                                wlocB[:, b:b + 1],
                                start=True,
                                stop=True,
                            )
                    nc.vector.tensor_copy(
                        XTr[:, HC:2 * HC, t, :],
                        ctxp[:].rearrange("p (k b) -> p k b", k=HC),
                    )
                    if rec_bf16:
                        nc.vector.tensor_copy(
                            XTBr[:, HC:2 * HC, t, :],
                            ctxp[:].rearrange("p (k b) -> p k b", k=HC),
                        )

                attention(0)

                for t in range(1, t_steps + 1):
                    # ---- GRU ----
                    # gi = W_ihc @ ctx(t-1); rz part accumulates gh too
                    gips = sp.tile([128, GM * BL], F32, tag="gips")
                    ghn = sp.tile([128, HC * BL], F32, tag="ghn")
                    for mt in range(GM):
                        o = mt * BL
                        rz = mt < 8
                        for kc in range(HC):
                            nc.tensor.matmul(
                                gips[:, o:o + BL],
                                wihc[:, kc * 3 * H + mt * 128:kc * 3 * H + (mt + 1) * 128],
                                XTBr[:, HC + kc, t - 1, :],
                                start=(kc == 0),
                                stop=(not rz and kc == HC - 1),
                            )
                        if rz:
                            for kc in range(HC):
                                nc.tensor.matmul(
                                    gips[:, o:o + BL],
                                    whh[:, kc * 3 * H + mt * 128:kc * 3 * H + (mt + 1) * 128],
                                    XTBr[:, kc, t - 1, :],
                                    start=False,
                                    stop=(kc == HC - 1),
                                )
                        else:
                            m2 = mt - 8
                            for kc in range(HC):
                                nc.tensor.matmul(
                                    ghn[:, m2 * BL:(m2 + 1) * BL],
                                    whh[:, kc * 3 * H + mt * 128:kc * 3 * H + (mt + 1) * 128],
                                    XTBr[:, kc, t - 1, :],
                                    start=(kc == 0),
                                    stop=(kc == HC - 1),
                                )
                    # rz_pre = gips[:, :32] + GX(t-1) rz part
                    rzp = wp.tile([128, 8 * BL], F32, tag="rzp")
                    nc.vector.tensor_add(
                        rzp[:].rearrange("p (m r) -> p m r", m=8),
                        gips[:, 0:8 * BL].rearrange("p (m r) -> p m r", m=8),
                        gxbr[:, 0:8, (t - 1) * BL:t * BL],
                    )
                    # rzt = tanh(rz_pre / 2); r = (rzt+1)/2 folded downstream
                    rzt = wp.tile([128, 8 * BL], F32, tag="rzt")
                    nc.scalar.activation(rzt[:], rzp[:], AF.Tanh, scale=0.5)
                    # X' = (gh_n + b_hh_n) * 0.5
                    xp = wp.tile([128, HC * BL], F32, tag="xp")
                    for m in range(HC):
                        nc.vector.tensor_scalar(
                            xp[:, m * BL:(m + 1) * BL],
                            ghn[:, m * BL:(m + 1) * BL],
                            bhhn[:, m:m + 1],
                            0.5,
                            op0=ALU.add,
                            op1=ALU.mult,
                        )
                    # inner = (tanh_r + 1) * X'
                    inner = wp.tile([128, HC * BL], F32, tag="inner")
                    nc.vector.scalar_tensor_tensor(
                        inner[:], rzt[:, 0:HC * BL], 1.0, xp[:],
                        op0=ALU.add, op1=ALU.mult,
                    )
                    # npre = (gi_n + GX_n) + inner
                    t1 = wp.tile([128, HC * BL], F32, tag="t1")
                    nc.vector.tensor_add(
                        t1[:].rearrange("p (m r) -> p m r", m=HC),
                        gips[:, 8 * BL:GM * BL].rearrange("p (m r) -> p m r", m=HC),
                        gxbr[:, 8:GM, (t - 1) * BL:t * BL],
                    )
                    npre = wp.tile([128, HC * BL], F32, tag="npre")
                    nc.vector.tensor_add(npre[:], t1[:], inner[:])
                    ngate = wp.tile([128, HC * BL], F32, tag="ngate")
                    nc.scalar.activation(ngate[:], npre[:], AF.Tanh)
                    # h_new = n + (tanh_z+1)/2 * (h - n)
                    d = wp.tile([128, HC * BL], F32, tag="d")
                    nc.vector.tensor_sub(
                        d[:].rearrange("p (m r) -> p m r", m=HC),
                        XTr[:, 0:HC, t - 1, :],
                        ngate[:].rearrange("p (m r) -> p m r", m=HC),
                    )
                    e = wp.tile([128, HC * BL], F32, tag="e")
                    nc.vector.scalar_tensor_tensor(
                        e[:], rzt[:, HC * BL:8 * BL], 1.0, d[:],
                        op0=ALU.add, op1=ALU.mult,
                    )
                    nc.vector.scalar_tensor_tensor(
                        XTr[:, 0:HC, t, :],
                        e[:].rearrange("p (m r) -> p m r", m=HC),
                        0.5,
                        ngate[:].rearrange("p (m r) -> p m r", m=HC),
                        op0=ALU.mult, op1=ALU.add,
                    )
                    if rec_bf16:
                        nc.vector.tensor_copy(XTBr[:, 0:HC, t, :],
                                              XTr[:, 0:HC, t, :])
                    # ---- attention on h_new ----
                    attention(t)

            # ================= outputs =================
            nc.sync.dma_start(d_hfin[:], XTr[:, 0:HC, t_steps, :])
            nc.sync.dma_start(d_attw[:], attT[:])
            if t_steps < T:
                nc.gpsimd.dma_start(d_xtdbg[:], XT[:])

            if with_head:
                with (
                    tc.tile_pool(name="hp", bufs=1, space="PSUM") as hp,
                    tc.tile_pool(name="whp", bufs=3) as whp,
                    tc.tile_pool(name="outp", bufs=4) as outp,
                ):
                    if head_bf16 and rec_bf16:
                        def xt_slice(kc, mt):
                            o = kc * (T + 1) * BL + BL
                            return XTB[:, o + mt * 128:o + (mt + 1) * 128]
                    elif head_bf16:
                        xth = cp.tile([128, KC2H * MT * 128], BF16)
                        for kc in range(KC2H):
                            o = kc * (T + 1) * BL + BL
                            nc.vector.tensor_copy(
                                xth[:, kc * MT * 128:(kc + 1) * MT * 128],
                                XT[:, o:o + MT * 128],
                            )

                        def xt_slice(kc, mt):
                            o = kc * MT * 128
                            return xth[:, o + mt * 128:o + (mt + 1) * 128]
                    else:
                        def xt_slice(kc, mt):
                            o = kc * (T + 1) * BL + BL
                            return XT[:, o + mt * 128:o + (mt + 1) * 128]

                    for blk in range((NB + NBLK - 1) // NBLK):
                        nb0 = blk * NBLK
                        nbs = min(NBLK, NB - nb0)
                        whb = whp.tile([128, NBLK * KC2H * 512], HDT, tag="whb")
                        nc.sync.dma_start(
                            whb[:].rearrange("p (c v) -> p c v", v=512)
                                  [:, 0:nbs * KC2H, :],
                            d_wht[nb0:nb0 + nbs].rearrange("n k p v -> p (n k) v"),
                        )
                        for nb in range(nbs):
                            ps = [hp.tile([128, 512], F32,
                                          tag=f"hps{(nb0 + nb) % NBLK}_{m}",
                                          name=f"hps_{nb0 + nb}_{m}")
                                  for m in range(MT)]
                            for kc in range(KC2H):
                                o = (nb * KC2H + kc) * 512
                                for mt in range(MT):
                                    nc.tensor.matmul(
                                        ps[mt][:],
                                        xt_slice(kc, mt),
                                        whb[:, o:o + 512],
                                        start=(kc == 0),
                                        stop=(kc == KC2H - 1),
                                    )
                            for mt in range(MT):
                                ot = outp.tile([128, 512], F32, tag="ot")
                                nc.vector.tensor_copy(ot[:], ps[mt][:])
                                nc.gpsimd.dma_start(
                                    d_logits[mt * 128:(mt + 1) * 128,
                                             (nb0 + nb) * 512:(nb0 + nb + 1) * 512],
                                    ot[:],
                                )
    nc.finalize()
    return nc


# ======================= host side =======================

def _prep_core_inputs(inputs, with_head=True, head_bf16=False, rec_bf16=False):
    enc = np.asarray(inputs["encoder_outputs"], np.float32)
    eh = np.asarray(inputs["encoder_hidden"], np.float32)
    idx = np.asarray(inputs["target_idx"])
    emb = np.asarray(inputs["emb"], np.float32)
    Wa = np.asarray(inputs["Wa"], np.float32)
    ba = np.asarray(inputs["ba"], np.float32)
    v = np.asarray(inputs["v"], np.float32)
    W_ih = np.asarray(inputs["W_ih"], np.float32)
    W_hh = np.asarray(inputs["W_hh"], np.float32)
    b_ih = np.asarray(inputs["b_ih"], np.float32)
    b_hh = np.asarray(inputs["b_hh"], np.float32)
    Wh = np.asarray(inputs["Wh"], np.float32)

    targets = emb[idx]  # [B, T, H]

    if rec_bf16:
        import ml_dtypes
        RNP = ml_dtypes.bfloat16
    else:
        RNP = np.float32

    C = np.ascontiguousarray

    def R(x):
        return C(x.astype(RNP))

    sones = np.zeros((BL, BL * SRC), np.float32)
    for b in range(BL):
        sones[b, b * SRC:(b + 1) * SRC] = 1.0

    shared = {
        "sones": None,  # filled below
        "iden": None,
        "wihxT": R(W_ih[:, :H].T),
        "wihcT": R(W_ih[:, H:].T),
        "whhT": R(W_hh.T),
        "wahT": R(Wa[:, :H].T),
        "waeT": R(Wa[:, H:].T),
        "vT": R(v[0].reshape(H, 1)),
        "baT": C(ba.reshape(HC, 128).T),
        "bfold": C(np.concatenate([(b_ih + b_hh)[:2 * H], b_ih[2 * H:]])
                   .reshape(GM, 128).T),
        "bhhn": C(b_hh[2 * H:].reshape(HC, 128).T),
    }
    shared["sones"] = R(sones)
    shared["iden"] = R(np.eye(128, dtype=np.float32))
    if with_head:
        whp = np.zeros((VP, 2 * H), np.float32)
        whp[:V] = Wh
        wht = whp.T.reshape(KC2H, 128, NB, 512).transpose(2, 0, 1, 3)
        if head_bf16:
            import ml_dtypes
            wht = wht.astype(ml_dtypes.bfloat16)
        shared["whtile"] = C(wht)

    maps = []
    for c in range(NCORES):
        bs = slice(c * BL, (c + 1) * BL)
        e = enc[bs]  # [BL, S, H]
        m = dict(shared)
        m["h0T"] = C(eh[0, bs].T)
        m["encT"] = R(e.transpose(2, 0, 1).reshape(H, BL * SRC))
        m["encsb"] = R(e)
        m["targT"] = R(targets[bs].transpose(2, 1, 0).reshape(H, BL * T))
        maps.append(m)
    return maps


def _unshard(results, inputs, with_head=True):
    bh = np.asarray(inputs["bh"], np.float32)
    outs = np.empty((B, T, V), np.float32)
    attn = np.empty((B, T, SRC), np.float32)
    hfin = np.empty((1, B, H), np.float32)
    for c, r in enumerate(results):
        bs = slice(c * BL, (c + 1) * BL)
        if with_head:
            lp = r["logitsP"].reshape(T, BL, VP)
            outs[bs] = lp[:, :, :V].transpose(1, 0, 2)
        attn[bs] = r["attW"].reshape(128, T, BL).transpose(2, 1, 0)
        hfin[0, bs] = (r["hfin"].reshape(128, HC, BL).transpose(2, 1, 0)
                       .reshape(BL, H))
    if with_head:
        outs += bh
    return outs, hfin, attn


_NC_CACHE = {}
HEAD_BF16 = False
REC_BF16 = False


def _get_nc(t_steps=T, with_head=True):
    key = (t_steps, with_head, HEAD_BF16, REC_BF16)
    if key not in _NC_CACHE:
        _NC_CACHE[key] = build_nc(t_steps, with_head, head_bf16=HEAD_BF16,
                                  rec_bf16=REC_BF16)
    return _NC_CACHE[key]


def _install_ntff_shim():
    """The agent image lacks ``antenv.axon_hooks``; provide it so
    run_bass_kernel_spmd(trace=True) can reach the NTFF profiler."""
    import sys
    import types
    if "antenv.axon_hooks" in sys.modules:
        return
    try:
        from trn_agent_boot.trn_boot import _ntff_profile_via_ctypes
        hook = _ntff_profile_via_ctypes("/opt/axon/libaxon_pjrt.so")
    except Exception:
        hook = None
    mod = types.ModuleType("antenv.axon_hooks")
    mod._hook = hook
    mod.set_axon_ntff_profile_hook = lambda h: setattr(mod, "_hook", h)
    mod.get_axon_ntff_profile_hook = lambda: mod._hook
    sys.modules["antenv.axon_hooks"] = mod


def run_hw(inputs, trace=False):
    nc = _get_nc()
    if trace:
        try:
            _install_ntff_shim()
        except Exception:
            trace = False
    maps = _prep_core_inputs(inputs, head_bf16=HEAD_BF16, rec_bf16=REC_BF16)
    res = run_bass_kernel_spmd(nc, maps, list(range(NCORES)), trace=trace)
    out = _unshard(res.results, inputs)
    return out, res


def kernel(**inputs):
    out, _ = run_hw(inputs, trace=False)
    return out


# revision 25
# speedup vs baseline: 1.1021x; 1.1021x over previous
"""Trainium2 Bass kernel for nn_BHW2AttnRNNDecoder.

Model (per reference.py):
  B=32, SRC=128, T=64, H=512, V=32000
  - embedding gather of targets
  - precomputable: enc_proj = enc @ Wa_e^T (time invariant)
                   GX = targets @ W_ih_x^T (depends only on embeddings)
  - sequential recurrence over T: GRU step + Bahdanau-style attention
  - big head GEMM logits = [h;ctx] @ Wh^T (+bh) batched over all T at the end

Sharding: pure data-parallel over batch across 8 cores (B_local=4/core),
no collectives. Params replicated. Head streams the (host-pre-tiled) Wh^T
from HBM. Host does embedding gather, transposes/layout prep, final
concat + bh add.

Layout convention on-chip: feature-on-partition ("transposed") so that
small-batch elementwise work uses all 128 lanes.

All tensors fp32.
"""

import os
import numpy as np

import concourse.bass as bass
import concourse.bacc as bacc
import concourse.mybir as mybir
from concourse import tile
from concourse.bass_utils import run_bass_kernel_spmd

F32 = mybir.dt.float32
AF = mybir.ActivationFunctionType
ALU = mybir.AluOpType

B, SRC, T, H, V = 32, 128, 64, 512, 32000
NCORES = 8
BL = B // NCORES          # 4 batches per core
HC = H // 128             # 4 h-chunks
GM = (3 * H) // 128       # 12 gate-chunks
KC2H = (2 * H) // 128     # 8 chunks of [h;ctx]
VP = 32256                # padded vocab (63 * 512)
NB = VP // 512            # 63 vocab tiles
NBLK = 4                  # vocab tiles per head block (PSUM: 2 Mt x 4 Nb = 8 banks)
MT = (BL * T) // 128      # 2 row-tiles for the head (256 rows)


def build_nc(t_steps=T, with_head=True, head_bf16=False, rec_bf16=False):
    BF16 = mybir.dt.bfloat16
    HDT = BF16 if head_bf16 else F32
    RDT = BF16 if rec_bf16 else F32
    nc = bacc.Bacc()

    # ---- I/O ----
    d_h0T = nc.declare_dram_parameter("h0T", [H, BL], F32, isOutput=False)
    d_encT = nc.declare_dram_parameter("encT", [H, BL * SRC], RDT, isOutput=False)
    d_encsb = nc.declare_dram_parameter("encsb", [BL, SRC, H], RDT, isOutput=False)
    d_targT = nc.declare_dram_parameter("targT", [H, BL * T], RDT, isOutput=False)
    d_wihxT = nc.declare_dram_parameter("wihxT", [H, 3 * H], RDT, isOutput=False)
    d_wihcT = nc.declare_dram_parameter("wihcT", [H, 3 * H], RDT, isOutput=False)
    d_whhT = nc.declare_dram_parameter("whhT", [H, 3 * H], RDT, isOutput=False)
    d_wahT = nc.declare_dram_parameter("wahT", [H, H], RDT, isOutput=False)
    d_waeT = nc.declare_dram_parameter("waeT", [H, H], RDT, isOutput=False)
    d_vT = nc.declare_dram_parameter("vT", [H, 1], RDT, isOutput=False)
    d_baT = nc.declare_dram_parameter("baT", [128, HC], F32, isOutput=False)
    d_sones = nc.declare_dram_parameter("sones", [BL, BL * SRC], RDT, isOutput=False)
    d_iden = nc.declare_dram_parameter("iden", [128, 128], RDT, isOutput=False)
    d_bfold = nc.declare_dram_parameter("bfold", [128, GM], F32, isOutput=False)
    d_bhhn = nc.declare_dram_parameter("bhhn", [128, HC], F32, isOutput=False)
    if with_head:
        d_wht = nc.declare_dram_parameter(
            "whtile", [NB, KC2H, 128, 512], HDT, isOutput=False
        )
        d_logits = nc.declare_dram_parameter(
            "logitsP", [BL * T, VP], F32, isOutput=True
        )
    d_attw = nc.declare_dram_parameter("attW", [128, T * BL], F32, isOutput=True)
    d_hfin = nc.declare_dram_parameter("hfin", [128, HC * BL], F32, isOutput=True)
    if t_steps < T:
        d_xtdbg = nc.declare_dram_parameter("xtdbg", [128, KC2H * (T + 1) * BL],
                                            F32, isOutput=True)

    with tile.TileContext(nc) as tc:
        with (
            tc.tile_pool(name="const", bufs=1) as cp,
            tc.tile_pool(name="work", bufs=2) as wp,
        ):
            # ---- load resident tensors ----
            wihc = cp.tile([128, HC * 3 * H], RDT)   # (kc, gate-col)
            nc.sync.dma_start(
                wihc[:].rearrange("p (k c) -> p k c", k=HC),
                d_wihcT[:].rearrange("(k p) c -> p k c", p=128),
            )
            whh = cp.tile([128, HC * 3 * H], RDT)
            nc.sync.dma_start(
                whh[:].rearrange("p (k c) -> p k c", k=HC),
                d_whhT[:].rearrange("(k p) c -> p k c", p=128),
            )
            wah = cp.tile([128, HC * H], RDT)
            nc.sync.dma_start(
                wah[:].rearrange("p (k c) -> p k c", k=HC),
                d_wahT[:].rearrange("(k p) c -> p k c", p=128),
            )
            vt = cp.tile([128, HC], RDT)
            nc.sync.dma_start(
                vt[:].rearrange("p (k o) -> p k o", k=HC),
                d_vT[:].rearrange("(k p) o -> p k o", p=128),
            )
            baT = cp.tile([128, HC], F32)
            nc.sync.dma_start(baT[:], d_baT[:])
            sones = cp.tile([BL, BL * SRC], RDT)
            nc.sync.dma_start(sones[:], d_sones[:])
            iden = cp.tile([128, 128], RDT)
            nc.sync.dma_start(iden[:], d_iden[:])
            bfold = cp.tile([128, GM], F32)
            nc.sync.dma_start(bfold[:], d_bfold[:])
            bhhn = cp.tile([128, HC], F32)
            nc.sync.dma_start(bhhn[:], d_bhhn[:])
            encsb = cp.tile([128, BL * H], RDT)      # (b, h), partition = s
            nc.sync.dma_start(
                encsb[:].rearrange("p (b h) -> p b h", b=BL),
                d_encsb[:].rearrange("b s h -> s b h"),
            )
            # big state buffers
            XT = cp.tile([128, KC2H * (T + 1) * BL], F32)  # (kc, t, b), t in 0..T
            XTr = XT[:].rearrange("p (k t b) -> p k t b", k=KC2H, b=BL)
            attT = cp.tile([128, T * BL], F32)             # (t, b), partition = s
            gxb = cp.tile([128, GM * T * BL], F32)         # (mt, (t,b))
            gxbr = gxb[:].rearrange("p (m c) -> p m c", m=GM)
            epjT = cp.tile([128, HC * BL * SRC], RDT)      # (hc, (b,s)), +ba
            if rec_bf16:
                XTB = cp.tile([128, KC2H * (T + 1) * BL], BF16)
                XTBr = XTB[:].rearrange("p (k t b) -> p k t b", k=KC2H, b=BL)
                attB = cp.tile([128, T * BL], BF16)
            else:
                XTB, XTBr, attB = XT, XTr, attT

            if t_steps < T:  # debug builds: zero-fill unwritten state
                nc.vector.memset(XT[:], 0.0)
                nc.vector.memset(attT[:], 0.0)
                if rec_bf16:
                    nc.vector.memset(XTB[:], 0.0)
                    nc.vector.memset(attB[:], 0.0)

            # h0 -> XT[:, 0:HC, 0, :]
            nc.sync.dma_start(
                XTr[:, 0:HC, 0, :],
                d_h0T[:].rearrange("(k p) b -> p k b", p=128),
            )
            if rec_bf16:
                nc.vector.tensor_copy(XTBr[:, 0:HC, 0, :], XTr[:, 0:HC, 0, :])

            # ================= precompute =================
            with (
                tc.tile_pool(name="pre_psum", bufs=2, space="PSUM") as pp,
                tc.tile_pool(name="pre_sbuf", bufs=1) as pre,
            ):
                wae = pre.tile([128, HC * H], RDT)
                nc.sync.dma_start(
                    wae[:].rearrange("p (k c) -> p k c", k=HC),
                    d_waeT[:].rearrange("(k p) c -> p k c", p=128),
                )
                wihx = pre.tile([128, HC * 3 * H], RDT)
                nc.sync.dma_start(
                    wihx[:].rearrange("p (k c) -> p k c", k=HC),
                    d_wihxT[:].rearrange("(k p) c -> p k c", p=128),
                )
                encT = pre.tile([128, HC * BL * SRC], RDT)  # (kc, (b,s))
                nc.sync.dma_start(
                    encT[:].rearrange("p (k c) -> p k c", k=HC),
                    d_encT[:].rearrange("(k p) c -> p k c", p=128),
                )
                targ = pre.tile([128, HC * BL * T], RDT)  # (kc, (t,b))
                nc.sync.dma_start(
                    targ[:].rearrange("p (k c) -> p k c", k=HC),
                    d_targT[:].rearrange("(k p) c -> p k c", p=128),
                )
                # enc_proj^T [hc, (b,s)] = Wa_e @ enc^T
                for mt in range(HC):
                    ep = pp.tile([128, BL * SRC], F32, tag="ep")
                    for kc in range(HC):
                        nc.tensor.matmul(
                            ep[:],
                            wae[:, kc * H + mt * 128:kc * H + (mt + 1) * 128],
                            encT[:, kc * BL * SRC:(kc + 1) * BL * SRC],
                            start=(kc == 0),
                            stop=(kc == HC - 1),
                        )
                    nc.vector.tensor_scalar_add(
                        epjT[:, mt * BL * SRC:(mt + 1) * BL * SRC], ep[:],
                        baT[:, mt:mt + 1],
                    )
                # GXb [mt, (t,b)] = W_ih_x @ targets^T + bias_fold
                for mt in range(GM):
                    gx = pp.tile([128, BL * T], F32, tag="gx")
                    for kc in range(HC):
                        nc.tensor.matmul(
                            gx[:],
                            wihx[:, kc * 3 * H + mt * 128:kc * 3 * H + (mt + 1) * 128],
                            targ[:, kc * BL * T:(kc + 1) * BL * T],
                            start=(kc == 0),
                            stop=(kc == HC - 1),
                        )
                    nc.vector.tensor_scalar(
                        gxb[:, mt * BL * T:(mt + 1) * BL * T],
                        gx[:],
                        bfold[:, mt:mt + 1],
                        None,
                        op0=ALU.add,
                    )

            # ================= recurrence =================
            with tc.tile_pool(name="sp", bufs=1, space="PSUM") as sp:

                def attention(t):
                    """Attention using h at XT[:, :, t, :]; writes ctx into
                    XT[:, HC:, t, :]; writes normalized w into attT col (t-1)
                    (skipped for t==0)."""
                    # q^T = Wa_h @ h : psum [128, (mt,b)]
                    qp = sp.tile([128, HC * BL], F32, tag="qp")
                    for mt in range(HC):
                        for kc in range(HC):
                            nc.tensor.matmul(
                                qp[:, mt * BL:(mt + 1) * BL],
                                wah[:, kc * H + mt * 128:kc * H + (mt + 1) * 128],
                                XTBr[:, kc, t, :],
                                start=(kc == 0),
                                stop=(kc == HC - 1),
                            )
                    # energy = tanh((encproj+ba) + q): 16 biased ACT ops
                    qba = wp.tile([128, HC * BL], F32, tag="qba")
                    nc.vector.tensor_copy(qba[:], qp[:])
                    ET = wp.tile([128, HC * BL * SRC], RDT, tag="ET")
                    for hc in range(HC):
                        for b in range(BL):
                            o = hc * BL * SRC + b * SRC
                            nc.scalar.activation(
                                ET[:, o:o + SRC],
                                epjT[:, o:o + SRC],
                                AF.Tanh,
                                bias=qba[:, hc * BL + b:hc * BL + b + 1],
                            )
                    scor = sp.tile([1, BL * SRC], F32, tag="scor")
                    for kc in range(HC):
                        nc.tensor.matmul(
                            scor[:],
                            vt[:, kc:kc + 1],
                            ET[:, kc * BL * SRC:(kc + 1) * BL * SRC],
                            start=(kc == 0),
                            stop=(kc == HC - 1),
                        )
                    # softmax (no max-sub; scores are O(1))
                    wexp = wp.tile([1, BL * SRC], F32, tag="wexp")
                    sums = wp.tile([1, BL], F32, tag="sums")
                    for b in range(BL):
                        nc.scalar.activation(
                            wexp[:, b * SRC:(b + 1) * SRC],
                            scor[:, b * SRC:(b + 1) * SRC],
                            AF.Exp,
                            accum_out=sums[:, b:b + 1],
                        )
                    recip = wp.tile([1, BL], F32, tag="recip")
                    nc.vector.reciprocal(recip[:], sums[:])
                    # wT normalized via K=1 matmul transpose trick
                    wtp = sp.tile([128, BL], F32, tag="wtp")
                    for b in range(BL):
                        nc.tensor.matmul(
                            wtp[:, b:b + 1],
                            wexp[:, b * SRC:(b + 1) * SRC],
                            recip[:, b:b + 1],
                            start=True,
                            stop=True,
                        )
                    if t == 0:
                        wsb = wp.tile([128, BL], F32, tag="wsb")
                        wloc = wsb[:]
                        if rec_bf16:
                            wsbB = wp.tile([128, BL], RDT, tag="wsbB")
                            wlocB = wsbB[:]
                        else:
                            wlocB = wloc
                    else:
                        wloc = attT[:, (t - 1) * BL:t * BL]
                        wlocB = attB[:, (t - 1) * BL:t * BL]
                    nc.vector.tensor_copy(wloc, wtp[:])
                    if rec_bf16:
                        nc.vector.tensor_copy(wlocB, wtp[:])
                    # ctx^T [hc, b] = enc_b^T @ w_b
                    ctxp = sp.tile([128, HC * BL], F32, tag="ctxp")
                    for hc in range(HC):
                        for b in range(BL):
                            nc.tensor.matmul(
                                ctxp[:, hc * BL + b:hc * BL + b + 1],
                                encsb[:, b * H + hc * 128:b * H + (hc + 1) * 128],
                                wlocB[:, b:b + 1],
                                start=True,
                                stop=True,
                            )
                    nc.vector.tensor_copy(
                        XTr[:, HC:2 * HC, t, :],
                        ctxp[:].rearrange("p (k b) -> p k b", k=HC),
                    )
                    if rec_bf16:
                        nc.vector.tensor_copy(
                            XTBr[:, HC:2 * HC, t, :],
                            ctxp[:].rearrange("p (k b) -> p k b", k=HC),
                        )

                attention(0)

                for t in range(1, t_steps + 1):
                    # ---- GRU ----
                    # gi = W_ihc @ ctx(t-1); rz part accumulates gh too
                    gips = sp.tile([128, GM * BL], F32, tag="gips")
                    ghn = sp.tile([128, HC * BL], F32, tag="ghn")
                    for mt in range(GM):
                        o = mt * BL
                        rz = mt < 8
                        for kc in range(HC):
                            nc.tensor.matmul(
                                gips[:, o:o + BL],
                                wihc[:, kc * 3 * H + mt * 128:kc * 3 * H + (mt + 1) * 128],
                                XTBr[:, HC + kc, t - 1, :],
                                start=(kc == 0),
                                stop=(not rz and kc == HC - 1),
                            )
                        if rz:
                            for kc in range(HC):
                                nc.tensor.matmul(
                                    gips[:, o:o + BL],
                                    whh[:, kc * 3 * H + mt * 128:kc * 3 * H + (mt + 1) * 128],
                                    XTBr[:, kc, t - 1, :],
                                    start=False,
                                    stop=(kc == HC - 1),
                                )
                        else:
                            m2 = mt - 8
                            for kc in range(HC):
                                nc.tensor.matmul(
                                    ghn[:, m2 * BL:(m2 + 1) * BL],
                                    whh[:, kc * 3 * H + mt * 128:kc * 3 * H + (mt + 1) * 128],
                                    XTBr[:, kc, t - 1, :],
                                    start=(kc == 0),
                                    stop=(kc == HC - 1),
                                )
                    # rz_pre = gips[:, :32] + GX(t-1) rz part
                    rzp = wp.tile([128, 8 * BL], F32, tag="rzp")
                    nc.vector.tensor_add(
                        rzp[:].rearrange("p (m r) -> p m r", m=8),
                        gips[:, 0:8 * BL].rearrange("p (m r) -> p m r", m=8),
                        gxbr[:, 0:8, (t - 1) * BL:t * BL],
                    )
                    # rzt = tanh(rz_pre / 2); r = (rzt+1)/2 folded downstream
                    rzt = wp.tile([128, 8 * BL], F32, tag="rzt")
                    nc.scalar.activation(rzt[:], rzp[:], AF.Tanh, scale=0.5)
                    # X' = (gh_n + b_hh_n) * 0.5
                    xp = wp.tile([128, HC * BL], F32, tag="xp")
                    for m in range(HC):
                        nc.vector.tensor_scalar(
                            xp[:, m * BL:(m + 1) * BL],
                            ghn[:, m * BL:(m + 1) * BL],
                            bhhn[:, m:m + 1],
                            0.5,
                            op0=ALU.add,
                            op1=ALU.mult,
                        )
                    # inner = (tanh_r + 1) * X'
                    inner = wp.tile([128, HC * BL], F32, tag="inner")
                    nc.vector.scalar_tensor_tensor(
                        inner[:], rzt[:, 0:HC * BL], 1.0, xp[:],
                        op0=ALU.add, op1=ALU.mult,
                    )
                    # npre = (gi_n + GX_n) + inner
                    t1 = wp.tile([128, HC * BL], F32, tag="t1")
                    nc.vector.tensor_add(
                        t1[:].rearrange("p (m r) -> p m r", m=HC),
                        gips[:, 8 * BL:GM * BL].rearrange("p (m r) -> p m r", m=HC),
                        gxbr[:, 8:GM, (t - 1) * BL:t * BL],
                    )
                    npre = wp.tile([128, HC * BL], F32, tag="npre")
                    nc.vector.tensor_add(npre[:], t1[:], inner[:])
                    ngate = wp.tile([128, HC * BL], F32, tag="ngate")
                    nc.scalar.activation(ngate[:], npre[:], AF.Tanh)
                    # h_new = n + (tanh_z+1)/2 * (h - n)
                    d = wp.tile([128, HC * BL], F32, tag="d")
                    nc.vector.tensor_sub(
                        d[:].rearrange("p (m r) -> p m r", m=HC),
                        XTr[:, 0:HC, t - 1, :],
                        ngate[:].rearrange("p (m r) -> p m r", m=HC),
                    )
                    e = wp.tile([128, HC * BL], F32, tag="e")
                    nc.vector.scalar_tensor_tensor(
                        e[:], rzt[:, HC * BL:8 * BL], 1.0, d[:],
                        op0=ALU.add, op1=ALU.mult,
                    )
                    nc.vector.scalar_tensor_tensor(
                        XTr[:, 0:HC, t, :],
                        e[:].rearrange("p (m r) -> p m r", m=HC),
                        0.5,
                        ngate[:].rearrange("p (m r) -> p m r", m=HC),
                        op0=ALU.mult, op1=ALU.add,
                    )
                    if rec_bf16:
                        nc.vector.tensor_copy(XTBr[:, 0:HC, t, :],
                                              XTr[:, 0:HC, t, :])
                    # ---- attention on h_new ----
                    attention(t)

            # ================= outputs =================
            nc.sync.dma_start(d_hfin[:], XTr[:, 0:HC, t_steps, :])
            nc.sync.dma_start(d_attw[:], attT[:])
            if t_steps < T:
                nc.gpsimd.dma_start(d_xtdbg[:], XT[:])

            if with_head:
                with (
                    tc.tile_pool(name="hp", bufs=1, space="PSUM") as hp,
                    tc.tile_pool(name="whp", bufs=3) as whp,
                    tc.tile_pool(name="outp", bufs=4) as outp,
                ):
                    if head_bf16 and rec_bf16:
                        def xt_slice(kc, mt):
                            o = kc * (T + 1) * BL + BL
                            return XTB[:, o + mt * 128:o + (mt + 1) * 128]
                    elif head_bf16:
                        xth = cp.tile([128, KC2H * MT * 128], BF16)
                        for kc in range(KC2H):
                            o = kc * (T + 1) * BL + BL
                            nc.vector.tensor_copy(
                                xth[:, kc * MT * 128:(kc + 1) * MT * 128],
                                XT[:, o:o + MT * 128],
                            )

                        def xt_slice(kc, mt):
                            o = kc * MT * 128
                            return xth[:, o + mt * 128:o + (mt + 1) * 128]
                    else:
                        def xt_slice(kc, mt):
                            o = kc * (T + 1) * BL + BL
                            return XT[:, o + mt * 128:o + (mt + 1) * 128]

                    for blk in range((NB + NBLK - 1) // NBLK):
                        nb0 = blk * NBLK
                        nbs = min(NBLK, NB - nb0)
                        whb = whp.tile([128, NBLK * KC2H * 512], HDT, tag="whb")
                        nc.sync.dma_start(
                            whb[:].rearrange("p (c v) -> p c v", v=512)
                                  [:, 0:nbs * KC2H, :],
                            d_wht[nb0:nb0 + nbs].rearrange("n k p v -> p (n k) v"),
                        )
                        for nb in range(nbs):
                            ps = [hp.tile([128, 512], F32,
                                          tag=f"hps{(nb0 + nb) % NBLK}_{m}",
                                          name=f"hps_{nb0 + nb}_{m}")
                                  for m in range(MT)]
                            for kc in range(KC2H):
                                o = (nb * KC2H + kc) * 512
                                for mt in range(MT):
                                    nc.tensor.matmul(
                                        ps[mt][:],
                                        xt_slice(kc, mt),
                                        whb[:, o:o + 512],
                                        start=(kc == 0),
                                        stop=(kc == KC2H - 1),
                                    )
                            for mt in range(MT):
                                ot = outp.tile([128, 512], F32, tag="ot")
                                nc.vector.tensor_copy(ot[:], ps[mt][:])
                                nc.gpsimd.dma_start(
                                    d_logits[mt * 128:(mt + 1) * 128,
                                             (nb0 + nb) * 512:(nb0 + nb + 1) * 512],
                                    ot[:],
                                )
    nc.finalize()
    return nc


# ======================= host side =======================

def _prep_core_inputs(inputs, with_head=True, head_bf16=False, rec_bf16=False):
    enc = np.asarray(inputs["encoder_outputs"], np.float32)
    eh = np.asarray(inputs["encoder_hidden"], np.float32)
    idx = np.asarray(inputs["target_idx"])
    emb = np.asarray(inputs["emb"], np.float32)
    Wa = np.asarray(inputs["Wa"], np.float32)
    ba = np.asarray(inputs["ba"], np.float32)
    v = np.asarray(inputs["v"], np.float32)
    W_ih = np.asarray(inputs["W_ih"], np.float32)
    W_hh = np.asarray(inputs["W_hh"], np.float32)
    b_ih = np.asarray(inputs["b_ih"], np.float32)
    b_hh = np.asarray(inputs["b_hh"], np.float32)
    Wh = np.asarray(inputs["Wh"], np.float32)

    targets = emb[idx]  # [B, T, H]

    if rec_bf16:
        import ml_dtypes
        RNP = ml_dtypes.bfloat16
    else:
        RNP = np.float32

    C = np.ascontiguousarray

    def R(x):
        return C(x.astype(RNP))

    sones = np.zeros((BL, BL * SRC), np.float32)
    for b in range(BL):
        sones[b, b * SRC:(b + 1) * SRC] = 1.0

    shared = {
        "sones": None,  # filled below
        "iden": None,
        "wihxT": R(W_ih[:, :H].T),
        "wihcT": R(W_ih[:, H:].T),
        "whhT": R(W_hh.T),
        "wahT": R(Wa[:, :H].T),
        "waeT": R(Wa[:, H:].T),
        "vT": R(v[0].reshape(H, 1)),
        "baT": C(ba.reshape(HC, 128).T),
        "bfold": C(np.concatenate([(b_ih + b_hh)[:2 * H], b_ih[2 * H:]])
                   .reshape(GM, 128).T),
        "bhhn": C(b_hh[2 * H:].reshape(HC, 128).T),
    }
    shared["sones"] = R(sones)
    shared["iden"] = R(np.eye(128, dtype=np.float32))
    if with_head:
        whp = np.zeros((VP, 2 * H), np.float32)
        whp[:V] = Wh
        wht = whp.T.reshape(KC2H, 128, NB, 512).transpose(2, 0, 1, 3)
        if head_bf16:
            import ml_dtypes
            wht = wht.astype(ml_dtypes.bfloat16)
        shared["whtile"] = C(wht)

    maps = []
    for c in range(NCORES):
        bs = slice(c * BL, (c + 1) * BL)
        e = enc[bs]  # [BL, S, H]
        m = dict(shared)
        m["h0T"] = C(eh[0, bs].T)
        m["encT"] = R(e.transpose(2, 0, 1).reshape(H, BL * SRC))
        m["encsb"] = R(e)
        m["targT"] = R(targets[bs].transpose(2, 1, 0).reshape(H, BL * T))
        maps.append(m)
    return maps


def _unshard(results, inputs, with_head=True):
    bh = np.asarray(inputs["bh"], np.float32)
    outs = np.empty((B, T, V), np.float32)
    attn = np.empty((B, T, SRC), np.float32)
    hfin = np.empty((1, B, H), np.float32)
    for c, r in enumerate(results):
        bs = slice(c * BL, (c + 1) * BL)
        if with_head:
            lp = r["logitsP"].reshape(T, BL, VP)
            outs[bs] = lp[:, :, :V].transpose(1, 0, 2)
        attn[bs] = r["attW"].reshape(128, T, BL).transpose(2, 1, 0)
        hfin[0, bs] = (r["hfin"].reshape(128, HC, BL).transpose(2, 1, 0)
                       .reshape(BL, H))
    if with_head:
        outs += bh
    return outs, hfin, attn


_NC_CACHE = {}
HEAD_BF16 = False
REC_BF16 = False


def _get_nc(t_steps=T, with_head=True):
    key = (t_steps, with_head, HEAD_BF16, REC_BF16)
    if key not in _NC_CACHE:
        _NC_CACHE[key] = build_nc(t_steps, with_head, head_bf16=HEAD_BF16,
                                  rec_bf16=REC_BF16)
    return _NC_CACHE[key]


def _install_ntff_shim():
    """The agent image lacks ``antenv.axon_hooks``; provide it so
    run_bass_kernel_spmd(trace=True) can reach the NTFF profiler."""
    import sys
    import types
    if "antenv.axon_hooks" in sys.modules:
        return
    try:
        from trn_agent_boot.trn_boot import _ntff_profile_via_ctypes
        hook = _ntff_profile_via_ctypes("/opt/axon/libaxon_pjrt.so")
    except Exception:
        hook = None
    mod = types.ModuleType("antenv.axon_hooks")
    mod._hook = hook
    mod.set_axon_ntff_profile_hook = lambda h: setattr(mod, "_hook", h)
    mod.get_axon_ntff_profile_hook = lambda: mod._hook
    sys.modules["antenv.axon_hooks"] = mod


def run_hw(inputs, trace=False):
    nc = _get_nc()
    if trace:
        try:
            _install_ntff_shim()
        except Exception:
            trace = False
    maps = _prep_core_inputs(inputs, head_bf16=HEAD_BF16, rec_bf16=REC_BF16)
    res = run_bass_kernel_spmd(nc, maps, list(range(NCORES)), trace=trace)
    out = _unshard(res.results, inputs)
    return out, res


def kernel(**inputs):
    out, _ = run_hw(inputs, trace=False)
    return out
